# revision 1
# baseline (speedup 1.0000x reference)
"""Multi-head causal attention (B=2, S=2048, D=1024, H=16, hd=64) on 8 TRN2
NeuronCores.

Sharding: 2-way batch x 4-way head tensor parallel. Core c handles batch
c//4 and heads 4*(c%4) .. 4*(c%4)+3 (a 256-column feature slice of the QKV
projections / 256 rows of o_W). Each core computes a full [2048, 1024]
partial of its batch's output; the host sums the 4 partials per batch.

All matmuls in fp16 (fp32 PSUM accumulate; validated end-to-end max rel
error ~5e-4 vs the fp32 reference):
  1. Qt/Kt feature-major:  Qt[f, tok] = sum_D wq[D, f] * xT[D, tok]
  2. V row-major (stationary operand = xT chunk), with a 1.0 column
     appended per head ([tok, 65] blocks) so the attention-value matmul
     also produces the softmax denominator.
  3. Causal attention per (head, 512-query chunk), keys in 128 chunks:
       St[k, q] = Kt_chunk.T @ Qt      (scores transposed)
       U = exp(St / 8)                 (ACT, scale fused; no max
                                        subtraction -- scores are O(1))
       causal mask: gpsimd affine_select zeroes k > q on the diagonal
       128x128 block; sub-diagonal key chunks are skipped entirely.
     AV row-major per 128-query subchunk, U slice as the stationary:
       AO_aug[q, 65] += U_i[:, qslice].T @ V_aug[i]   (col 64 = denom)
     Normalize: rec = 1/AO_aug[:, 64] (DVE [128,1]), then
     tensor_scalar multiply (per-partition broadcast).
  4. AO transposed back to feature-major via PE transpose (fp16), then
     y[tok, :] = sum_f AOt[f-chunk, tok].T @ wo[f-chunk, :]
"""

import numpy as np

import concourse.mybir as mybir
import concourse.tile as tile
from concourse import bacc
from concourse.bass_utils import run_bass_kernel_spmd

F32 = mybir.dt.float32
F16 = mybir.dt.float16

S = 2048          # tokens per batch (= per core)
D = 1024          # model dim
HD = 64           # head dim
CORE_HEADS = 4    # heads per core
CF = CORE_HEADS * HD  # feature columns per core (256)
QC = 512          # query chunk (QK/exp granularity)
KC = 128          # key chunk
NQ = S // QC      # 4 query chunks
NK = S // KC      # 16 key chunks
ND = D // 128     # 8 contraction chunks

_CACHE = {}


def build_nc():
    nc = bacc.Bacc()
    xT = nc.dram_tensor("xT", [D, S], F16, kind="ExternalInput")
    wq = nc.dram_tensor("wq", [D, CF], F16, kind="ExternalInput")
    wk = nc.dram_tensor("wk", [D, CF], F16, kind="ExternalInput")
    wv = nc.dram_tensor("wv", [D, CF], F16, kind="ExternalInput")
    wo = nc.dram_tensor("wo", [CF, D], F16, kind="ExternalInput")
    y = nc.dram_tensor("y", [S, D], F32, kind="ExternalOutput")

    with tile.TileContext(nc) as tc:
        with (
            tc.tile_pool(name="big", bufs=1) as big,
            tc.tile_pool(name="w", bufs=1) as wpool,
            tc.tile_pool(name="u", bufs=34) as upool,
            tc.tile_pool(name="aoq", bufs=16) as aoqpool,
            tc.tile_pool(name="aot", bufs=3) as aotpool,
            tc.tile_pool(name="sm", bufs=8) as smpool,
            tc.tile_pool(name="ost", bufs=4) as ostpool,
            tc.tile_pool(name="ps", bufs=2, space="PSUM") as psp,
            tc.tile_pool(name="pav", bufs=2, space="PSUM") as pavp,
            tc.tile_pool(name="pt", bufs=2, space="PSUM") as ptp,
        ):
            # ---- constants ----
            ident = wpool.tile([128, 128], F16, tag="ident")
            nc.gpsimd.memset(ident[:], 0.0)
            nc.gpsimd.affine_select(
                out=ident[:], in_=ident[:],
                compare_op=mybir.AluOpType.not_equal, fill=1.0,
                base=0, channel_multiplier=1, pattern=[[-1, 128]],
            )

            # ---- weight + activation loads ----
            wq_sb = wpool.tile([128, ND, CF], F16, tag="wq")
            wk_sb = wpool.tile([128, ND, CF], F16, tag="wk")
            wv_sb = wpool.tile([128, ND, CF], F16, tag="wv")
            wo_sb = wpool.tile([128, 2, D], F16, tag="wo")
            # Inputs streamed through both HWDGE issue engines (sync +
            # scalar) so the startup ramp isn't bound by one queue set.
            xs = big.tile([128, ND, S], F16, tag="xs")
            xTv = xT.rearrange("(n p) m -> p n m", p=128)
            wqv = wq.rearrange("(n p) m -> p n m", p=128)
            wkv = wk.rearrange("(n p) m -> p n m", p=128)
            for k in range(ND):
                nc.sync.dma_start(wq_sb[:, k, :], wqv[:, k, :])
                nc.scalar.dma_start(wk_sb[:, k, :], wkv[:, k, :])
                nc.sync.dma_start(xs[:, k, 0:S // 2], xTv[:, k, 0:S // 2])
                nc.scalar.dma_start(xs[:, k, S // 2:], xTv[:, k, S // 2:])

            nc.sync.dma_start(wv_sb[:], wv.rearrange("(n p) m -> p n m", p=128))
            nc.scalar.dma_start(wo_sb[:], wo.rearrange("(b p) n -> p b n", p=128))

            # ---- Q/K/V projections, interleaved per token chunk ----
            # qt/kt: [128, 2, S]: partition = feat % 128 (2 heads), block =
            # feat // 128 (head pair), col = token.
            qt = big.tile([128, 2, S], F16, tag="qt")
            kt = big.tile([128, 2, S], F16, tag="kt")
            v_sb = big.tile([128, NK, CORE_HEADS * (HD + 1)], F16, tag="v")
            nc.vector.memset(
                v_sb[:].rearrange("p n (h c) -> p n h c", c=HD + 1)[:, :, :, HD:],
                1.0,
            )
            for t in range(NQ):
                for f in range(2):
                    ps_q = ptp.tile([128, QC], F32, tag="t", name=f"pq{t}_{f}")
                    ps_k = ptp.tile([128, QC], F32, tag="t", name=f"pk{t}_{f}")
                    with nc.named_scope("mm_projqk"):
                        for k in range(ND):
                            nc.tensor.matmul(
                                ps_q[:],
                                wq_sb[:, k, 128 * f:128 * (f + 1)],
                                xs[:, k, QC * t:QC * (t + 1)],
                                start=(k == 0), stop=(k == ND - 1),
                            )
                            nc.tensor.matmul(
                                ps_k[:],
                                wk_sb[:, k, 128 * f:128 * (f + 1)],
                                xs[:, k, QC * t:QC * (t + 1)],
                                start=(k == 0), stop=(k == ND - 1),
                            )
                    nc.vector.tensor_copy(qt[:, f, QC * t:QC * (t + 1)], ps_q[:])
                    nc.vector.tensor_copy(kt[:, f, QC * t:QC * (t + 1)], ps_k[:])
                for tt in range(4 * t, 4 * t + 4):
                    ps = ptp.tile([128, CF], F32, tag="t")
                    with nc.named_scope("mm_projv"):
                        for k in range(ND):
                            nc.tensor.matmul(
                                ps[:],
                                xs[:, k, KC * tt:KC * (tt + 1)],
                                wv_sb[:, k, :],
                                start=(k == 0), stop=(k == ND - 1),
                            )
                    nc.vector.tensor_copy(
                        v_sb[:, tt, :].rearrange("p (h c) -> p h c", c=HD + 1)[:, :, :HD],
                        ps[:].rearrange("p (h c) -> p h c", c=HD),
                    )

            # ---- attention + output projection ----
            # Software-pipelined across (query-chunk, head-pair) units: the
            # AV matmuls of unit k-1 (pure PE work) are interleaved with the
            # QK+exp phase of unit k (ACT-paced) so the PE never waits on
            # the scalar engine.
            units = [(j, pair) for j in range(NQ) for pair in range(2)]
            us = {}
            ao_q = {}
            av_t = {}

            def emit_A(unit, i):
                # Both heads' scores land in one 2-bank PSUM tile so a
                # single ACTIVATE (and a single affine_select) covers the
                # pair — halves the fixed 352-cycle ACT pipeline overhead.
                j, pair = unit
                t = i - 4 * j
                qo = max(0, KC * t)
                w = QC - qo
                ps_s = psp.tile([128, 2, QC], F32, tag="s", name=f"s{j}_{pair}_{i}")
                for hx, h in enumerate((2 * pair, 2 * pair + 1)):
                    hp = 64 * (h % 2)
                    with nc.named_scope("mm_qk"):
                        nc.tensor.matmul(
                            ps_s[:, hx, 0:w],
                            kt[hp:hp + 64, pair, KC * i:KC * (i + 1)],
                            qt[hp:hp + 64, pair, QC * j + qo:QC * (j + 1)],
                            start=True, stop=True,
                            skip_group_check=True,
                        )
                u = upool.tile([128, 2, w], F16, tag="u", name=f"u{j}_{pair}_{i}")
                nc.scalar.activation(
                    u[:], ps_s[:, :, 0:w],
                    mybir.ActivationFunctionType.Exp, scale=0.125,
                )
                if t >= 0:
                    nc.gpsimd.affine_select(
                        out=u[:, :, 0:KC], in_=u[:, :, 0:KC],
                        compare_op=mybir.AluOpType.is_ge, fill=0.0,
                        base=0, channel_multiplier=-1,
                        pattern=[[0, 2], [1, KC]],
                    )
                us[unit, i] = (u, qo)

            def emit_B(unit):
                """Generator: AV matmuls for one unit, yielding after each
                key-chunk step; norms emitted at each query-subchunk's end.
                One PSUM bank per accumulation group (bank-granular
                start/stop semantics)."""
                j, pair = unit
                nk = 4 * j + 4
                if j not in ao_q:
                    ao_q[j] = [aoqpool.tile([128, CF], F16, tag="aoq",
                                            name=f"ao_q{j}_{qq}")
                               for qq in range(4)]
                for hx, h in enumerate((2 * pair, 2 * pair + 1)):
                    for qq in range(4):
                        av = pavp.tile([128, HD + 1], F32, tag="av",
                                       name=f"av{j}_{h}_{qq}")
                        last = 4 * j + qq
                        with nc.named_scope("mm_av"):
                            for i in range(last + 1):
                                u, qo = us[unit, i]
                                nc.tensor.matmul(
                                    av[:],
                                    u[:, hx, KC * qq - qo:KC * (qq + 1) - qo],
                                    v_sb[:, i, 65 * h:65 * h + 65],
                                    start=(i == 0), stop=(i == last),
                                )
                                yield
                        rec = smpool.tile([128, 1], F32, tag="rec",
                                          name=f"rec{j}_{h}_{qq}")
                        nc.vector.reciprocal(rec[:], av[:, HD:HD + 1])
                        nc.vector.tensor_scalar_mul(
                            ao_q[j][qq][:, HD * h:HD * (h + 1)],
                            av[:, 0:HD], rec[:],
                        )
                    yield
                for i in range(nk):
                    us.pop((unit, i), None)

            def emit_out(j):
                aot = aotpool.tile([128, 2, QC], F16, tag="aot", name=f"aot{j}")
                for qq in range(4):
                    for b in range(2):
                        ps_t = ptp.tile([128, 128], F16, tag="t",
                                        name=f"pt{j}_{qq}_{b}")
                        with nc.named_scope("mm_tpose"):
                            nc.tensor.transpose(
                                ps_t[:], ao_q[j][qq][:, 128 * b:128 * (b + 1)],
                                ident[:],
                            )
                        nc.vector.tensor_copy(
                            aot[:, b, KC * qq:KC * (qq + 1)], ps_t[:],
                        )
                for tt4 in range(QC // KC):
                    ps_o = [ptp.tile([128, QC], F32, tag="t",
                                     name=f"po{j}_{tt4}_{n}")
                            for n in range(2)]
                    with nc.named_scope("mm_oproj"):
                        for b in range(2):
                            for n in range(2):
                                nc.tensor.matmul(
                                    ps_o[n][:],
                                    aot[:, b, KC * tt4:KC * (tt4 + 1)],
                                    wo_sb[:, b, 512 * n:512 * (n + 1)],
                                    start=(b == 0), stop=(b == 1),
                                )
                    for n in range(2):
                        ost = ostpool.tile([128, QC], F32, tag="ost",
                                           name=f"ost{j}_{tt4}_{n}")
                        nc.vector.tensor_copy(ost[:], ps_o[n][:])
                        nc.sync.dma_start(
                            y[QC * j + KC * tt4:QC * j + KC * (tt4 + 1),
                              512 * n:512 * (n + 1)],
                            ost[:],
                        )

            prev_gen = None
            prev_unit = None
            for unit in units:
                j, pair = unit
                nk_c = 4 * j + 4
                for ii in range(nk_c):
                    emit_A(unit, ii)
                    if prev_gen is not None:
                        for _ in range(2):
                            next(prev_gen, None)
                if prev_gen is not None:
                    for _ in prev_gen:
                        pass
                    if prev_unit[1] == 1:
                        emit_out(prev_unit[0])
                prev_gen = emit_B(unit)
                prev_unit = unit
            for _ in prev_gen:
                pass
            emit_out(prev_unit[0])
    nc.compile()
    return nc


def _get_nc():
    if "nc" not in _CACHE:
        _CACHE["nc"] = build_nc()
    return _CACHE["nc"]


def make_in_maps(x, q_W, k_W, v_W, o_W):
    x = np.asarray(x, dtype=np.float32)
    in_maps = []
    xTs = [np.ascontiguousarray(x[b].T).astype(np.float16) for b in range(2)]
    for c in range(8):
        b, g = c // 4, c % 4
        fs = slice(CF * g, CF * (g + 1))
        in_maps.append({
            "xT": xTs[b],
            "wq": np.ascontiguousarray(np.asarray(q_W, np.float32)[fs].T).astype(np.float16),
            "wk": np.ascontiguousarray(np.asarray(k_W, np.float32)[fs].T).astype(np.float16),
            "wv": np.ascontiguousarray(np.asarray(v_W, np.float32)[fs].T).astype(np.float16),
            "wo": np.ascontiguousarray(np.asarray(o_W, np.float32)[:, fs].T).astype(np.float16),
        })
    return in_maps


def kernel(x, q_W, k_W, v_W, o_W, trace=False):
    nc = _get_nc()
    in_maps = make_in_maps(x, q_W, k_W, v_W, o_W)
    res = run_bass_kernel_spmd(nc, in_maps, core_ids=list(range(8)),
                               trace=trace)
    _CACHE["last_results"] = res
    ys = [res.results[c]["y"] for c in range(8)]
    out = np.stack([
        ys[0] + ys[1] + ys[2] + ys[3],
        ys[4] + ys[5] + ys[6] + ys[7],
    ]).astype(np.float32)
    return out



# revision 3
# speedup vs baseline: 1.0852x; 1.0852x over previous
"""Multi-head causal attention (B=2, S=2048, D=1024, H=16, hd=64) on 8 TRN2
NeuronCores.

Sharding: 2-way batch x 4-way head tensor parallel. Core c handles batch
c//4 and heads 4*(c%4) .. 4*(c%4)+3 (a 256-column feature slice of the QKV
projections / 256 rows of o_W). Each core computes a full [2048, 1024]
partial of its batch's output; the host sums the 4 partials per batch.

v2 structure (all matmuls fp16, fp32 PSUM accumulate):
  - Input DMA split across sync+gpsimd+vector queues, ordered so the
    first Q-projection tile's operands (wq + x tokens 0:512) land first.
  - Q/K/V projection for token chunk t is emitted interleaved into the
    attention stream of query chunk t-1, so the ACT engine (exp) gets
    work within ~5us of kernel start and the PE never drains.
  - Attention per (query chunk j, head pair): scores transposed
    St[k, q] = Kt.T @ Qt, exp via ACT (scale fused), causal mask via
    gpsimd affine_select, AV row-major with a ones-column appended to V
    so the same matmul produces the softmax denominator.
  - Output projection from PE-transposed AO; y stored fp16 (the host
    sums 4 fp16 partials per batch in fp32).
  - Optionally (U8=True) the exp output u is stored fp8e4 (exp shifted
    by -2ln2 to fit e4m3 range; the shift cancels in the softmax ratio)
    which halves the AV stationary LDWEIGHTS traffic.
"""

import numpy as np

import concourse.mybir as mybir
import concourse.tile as tile
from concourse import bacc
from concourse.bass_utils import run_bass_kernel_spmd

F32 = mybir.dt.float32
F16 = mybir.dt.float16
F8 = mybir.dt.float8e4

S = 2048          # tokens per batch (= per core)
D = 1024          # model dim
HD = 64           # head dim
CORE_HEADS = 4    # heads per core
CF = CORE_HEADS * HD  # feature columns per core (256)
QC = 512          # query chunk (QK/exp granularity)
KC = 128          # key chunk
NQ = S // QC      # 4 query chunks
NK = S // KC      # 16 key chunks
ND = D // 128     # 8 contraction chunks

U8 = False        # fp8e4 exp output (AV stationary)

_CACHE = {}


def build_nc(u8=U8):
    udt = F8 if u8 else F16
    ubias = float(-2.0 * np.log(2.0)) if u8 else 0.0
    nc = bacc.Bacc()
    xT = nc.dram_tensor("xT", [D, S], F16, kind="ExternalInput")
    wq = nc.dram_tensor("wq", [D, CF], F16, kind="ExternalInput")
    wk = nc.dram_tensor("wk", [D, CF], F16, kind="ExternalInput")
    wv = nc.dram_tensor("wv", [D, CF], F16, kind="ExternalInput")
    wo = nc.dram_tensor("wo", [CF, D], F16, kind="ExternalInput")
    y = nc.dram_tensor("y", [S, D], F16, kind="ExternalOutput")

    with tile.TileContext(nc) as tc:
        with (
            tc.tile_pool(name="big", bufs=1) as big,
            tc.tile_pool(name="w", bufs=1) as wpool,
            tc.tile_pool(name="u", bufs=34) as upool,
            tc.tile_pool(name="aoq", bufs=16) as aoqpool,
            tc.tile_pool(name="aot", bufs=3) as aotpool,
            tc.tile_pool(name="sm", bufs=8) as smpool,
            tc.tile_pool(name="ost", bufs=4) as ostpool,
            tc.tile_pool(name="ps", bufs=2, space="PSUM") as psp,
            tc.tile_pool(name="pav", bufs=2, space="PSUM") as pavp,
            tc.tile_pool(name="pt", bufs=2, space="PSUM") as ptp,
        ):
            # ---- constants ----
            ident = wpool.tile([128, 128], F16, tag="ident")
            nc.gpsimd.memset(ident[:], 0.0)
            nc.gpsimd.affine_select(
                out=ident[:], in_=ident[:],
                compare_op=mybir.AluOpType.not_equal, fill=1.0,
                base=0, channel_multiplier=1, pattern=[[-1, 128]],
            )

            # ---- weight + activation loads ----
            # Ordered so Q proj of token chunk 0 can start first: wq[k] and
            # xs[k, 0:512] alternate between the sync and gpsimd queues,
            # then wk/wv, then the remaining x token chunks on vector.
            wq_sb = wpool.tile([128, ND, CF], F16, tag="wq")
            wk_sb = wpool.tile([128, ND, CF], F16, tag="wk")
            wv_sb = wpool.tile([128, ND, CF], F16, tag="wv")
            wo_sb = wpool.tile([128, 2, D], F16, tag="wo")
            xs = big.tile([128, ND, S], F16, tag="xs")
            xTv = xT.rearrange("(n p) m -> p n m", p=128)
            wqv = wq.rearrange("(n p) m -> p n m", p=128)
            wkv = wk.rearrange("(n p) m -> p n m", p=128)
            wvv = wv.rearrange("(n p) m -> p n m", p=128)

            def q_eng(k):
                return nc.sync if k % 2 == 0 else nc.gpsimd

            for k in range(ND):
                q_eng(k).dma_start(wq_sb[:, k, :], wqv[:, k, :])
                q_eng(k + 1).dma_start(xs[:, k, 0:QC], xTv[:, k, 0:QC])
            for k in range(ND):
                q_eng(k).dma_start(wk_sb[:, k, :], wkv[:, k, :])
                q_eng(k + 1).dma_start(wv_sb[:, k, :], wvv[:, k, :])
            for k in range(ND):
                q_eng(k).dma_start(xs[:, k, QC:2 * QC], xTv[:, k, QC:2 * QC])
                q_eng(k + 1).dma_start(xs[:, k, 2 * QC:3 * QC],
                                       xTv[:, k, 2 * QC:3 * QC])
                q_eng(k).dma_start(xs[:, k, 3 * QC:], xTv[:, k, 3 * QC:])
            nc.gpsimd.dma_start(wo_sb[:], wo.rearrange("(b p) n -> p b n", p=128))

            # ---- Q/K/V projections ----
            # qt/kt: [128, 2, S]: partition = feat % 128 (2 heads), block =
            # feat // 128 (head pair), col = token.
            qt = big.tile([128, 2, S], F16, tag="qt")
            kt = big.tile([128, 2, S], F16, tag="kt")
            v_sb = big.tile([128, NK, CORE_HEADS * (HD + 1)], F16, tag="v")
            nc.vector.memset(
                v_sb[:].rearrange("p n (h c) -> p n h c", c=HD + 1)[:, :, :, HD:],
                1.0,
            )

            def emit_proj(t):
                """Generator: QKV projection matmuls for token chunk t."""
                for f in range(2):
                    ps_q = ptp.tile([128, QC], F32, tag="t", name=f"pq{t}_{f}")
                    ps_k = ptp.tile([128, QC], F32, tag="t", name=f"pk{t}_{f}")
                    with nc.named_scope("mm_projqk"):
                        for k in range(ND):
                            nc.tensor.matmul(
                                ps_q[:],
                                wq_sb[:, k, 128 * f:128 * (f + 1)],
                                xs[:, k, QC * t:QC * (t + 1)],
                                start=(k == 0), stop=(k == ND - 1),
                            )
                    yield
                    nc.vector.tensor_copy(qt[:, f, QC * t:QC * (t + 1)], ps_q[:])
                    with nc.named_scope("mm_projqk"):
                        for k in range(ND):
                            nc.tensor.matmul(
                                ps_k[:],
                                wk_sb[:, k, 128 * f:128 * (f + 1)],
                                xs[:, k, QC * t:QC * (t + 1)],
                                start=(k == 0), stop=(k == ND - 1),
                            )
                    yield
                    nc.vector.tensor_copy(kt[:, f, QC * t:QC * (t + 1)], ps_k[:])
                for tt in range(4 * t, 4 * t + 4):
                    ps = ptp.tile([128, CF], F32, tag="t")
                    with nc.named_scope("mm_projv"):
                        for k in range(ND):
                            nc.tensor.matmul(
                                ps[:],
                                xs[:, k, KC * tt:KC * (tt + 1)],
                                wv_sb[:, k, :],
                                start=(k == 0), stop=(k == ND - 1),
                            )
                    yield
                    nc.vector.tensor_copy(
                        v_sb[:, tt, :].rearrange("p (h c) -> p h c", c=HD + 1)[:, :, :HD],
                        ps[:].rearrange("p (h c) -> p h c", c=HD),
                    )

            # ---- attention + output projection ----
            # Software-pipelined across (query-chunk, head-pair) units: the
            # AV matmuls of unit k-1 (pure PE work) are interleaved with the
            # QK+exp phase of unit k (ACT-paced) so the PE never waits on
            # the scalar engine. The projection of token chunk j+1 is pumped
            # into the same stream.
            units = [(j, pair) for j in range(NQ) for pair in range(2)]
            us = {}
            ao_q = {}

            def emit_A(unit, i):
                # Both heads' scores land in one 2-bank PSUM tile so a
                # single ACTIVATE (and a single affine_select) covers the
                # pair — halves the fixed ACT pipeline overhead.
                j, pair = unit
                t = i - 4 * j
                qo = max(0, KC * t)
                w = QC - qo
                ps_s = psp.tile([128, 2, QC], F32, tag="s", name=f"s{j}_{pair}_{i}")
                for hx, h in enumerate((2 * pair, 2 * pair + 1)):
                    hp = 64 * (h % 2)
                    with nc.named_scope("mm_qk"):
                        nc.tensor.matmul(
                            ps_s[:, hx, 0:w],
                            kt[hp:hp + 64, pair, KC * i:KC * (i + 1)],
                            qt[hp:hp + 64, pair, QC * j + qo:QC * (j + 1)],
                            start=True, stop=True,
                            skip_group_check=True,
                        )
                u = upool.tile([128, 2, w], udt, tag="u", name=f"u{j}_{pair}_{i}")
                nc.scalar.activation(
                    u[:], ps_s[:, :, 0:w],
                    mybir.ActivationFunctionType.Exp, scale=0.125, bias=ubias,
                )
                if t >= 0:
                    nc.gpsimd.affine_select(
                        out=u[:, :, 0:KC], in_=u[:, :, 0:KC],
                        compare_op=mybir.AluOpType.is_ge, fill=0.0,
                        base=0, channel_multiplier=-1,
                        pattern=[[0, 2], [1, KC]],
                    )
                us[unit, i] = (u, qo)

            def emit_B(unit):
                """Generator: AV matmuls for one unit, yielding after each
                key-chunk step; norms emitted at each query-subchunk's end.
                One PSUM bank per accumulation group (bank-granular
                start/stop semantics)."""
                j, pair = unit
                nk = 4 * j + 4
                if j not in ao_q:
                    ao_q[j] = [aoqpool.tile([128, CF], F16, tag="aoq",
                                            name=f"ao_q{j}_{qq}")
                               for qq in range(4)]
                for hx, h in enumerate((2 * pair, 2 * pair + 1)):
                    for qq in range(4):
                        av = pavp.tile([128, HD + 1], F32, tag="av",
                                       name=f"av{j}_{h}_{qq}")
                        last = 4 * j + qq
                        with nc.named_scope("mm_av"):
                            for i in range(last + 1):
                                u, qo = us[unit, i]
                                nc.tensor.matmul(
                                    av[:],
                                    u[:, hx, KC * qq - qo:KC * (qq + 1) - qo],
                                    v_sb[:, i, 65 * h:65 * h + 65],
                                    start=(i == 0), stop=(i == last),
                                )
                                yield
                        rec = smpool.tile([128, 1], F32, tag="rec",
                                          name=f"rec{j}_{h}_{qq}")
                        nc.vector.reciprocal(rec[:], av[:, HD:HD + 1])
                        nc.vector.tensor_scalar_mul(
                            ao_q[j][qq][:, HD * h:HD * (h + 1)],
                            av[:, 0:HD], rec[:],
                        )
                    yield
                for i in range(nk):
                    us.pop((unit, i), None)

            def emit_out(j):
                aot = aotpool.tile([128, 2, QC], F16, tag="aot", name=f"aot{j}")
                for qq in range(4):
                    for b in range(2):
                        ps_t = ptp.tile([128, 128], F16, tag="t",
                                        name=f"pt{j}_{qq}_{b}")
                        with nc.named_scope("mm_tpose"):
                            nc.tensor.transpose(
                                ps_t[:], ao_q[j][qq][:, 128 * b:128 * (b + 1)],
                                ident[:],
                            )
                        nc.vector.tensor_copy(
                            aot[:, b, KC * qq:KC * (qq + 1)], ps_t[:],
                        )
                for tt4 in range(QC // KC):
                    ps_o = [ptp.tile([128, QC], F32, tag="t",
                                     name=f"po{j}_{tt4}_{n}")
                            for n in range(2)]
                    with nc.named_scope("mm_oproj"):
                        for b in range(2):
                            for n in range(2):
                                nc.tensor.matmul(
                                    ps_o[n][:],
                                    aot[:, b, KC * tt4:KC * (tt4 + 1)],
                                    wo_sb[:, b, 512 * n:512 * (n + 1)],
                                    start=(b == 0), stop=(b == 1),
                                )
                    for n in range(2):
                        ost = ostpool.tile([128, QC], F16, tag="ost",
                                           name=f"ost{j}_{tt4}_{n}")
                        nc.vector.tensor_copy(ost[:], ps_o[n][:])
                        q_eng(tt4 + n).dma_start(
                            y[QC * j + KC * tt4:QC * j + KC * (tt4 + 1),
                              512 * n:512 * (n + 1)],
                            ost[:],
                        )

            def pump(gen):
                if gen is not None and next(gen, "done") == "done":
                    return None
                return gen

            # token chunk 0 projection runs standalone; chunk j+1 is pumped
            # into query chunk j's attention stream.
            for _ in emit_proj(0):
                pass

            prev_gen = None
            prev_unit = None
            proj_gen = None
            for unit in units:
                j, pair = unit
                if pair == 0 and j + 1 < NQ:
                    proj_gen = emit_proj(j + 1)
                nk_c = 4 * j + 4
                for ii in range(nk_c):
                    emit_A(unit, ii)
                    proj_gen = pump(proj_gen)
                    if prev_gen is not None:
                        for _ in range(2):
                            next(prev_gen, None)
                if prev_gen is not None:
                    for _ in prev_gen:
                        proj_gen = pump(proj_gen)
                    if prev_unit[1] == 1:
                        emit_out(prev_unit[0])
                prev_gen = emit_B(unit)
                prev_unit = unit
            for _ in prev_gen:
                proj_gen = pump(proj_gen)
            emit_out(prev_unit[0])
    nc.compile()
    return nc


def _get_nc():
    if "nc" not in _CACHE:
        _CACHE["nc"] = build_nc()
    return _CACHE["nc"]


def make_in_maps(x, q_W, k_W, v_W, o_W):
    x = np.asarray(x, dtype=np.float32)
    in_maps = []
    xTs = [np.ascontiguousarray(x[b].T).astype(np.float16) for b in range(2)]
    for c in range(8):
        b, g = c // 4, c % 4
        fs = slice(CF * g, CF * (g + 1))
        in_maps.append({
            "xT": xTs[b],
            "wq": np.ascontiguousarray(np.asarray(q_W, np.float32)[fs].T).astype(np.float16),
            "wk": np.ascontiguousarray(np.asarray(k_W, np.float32)[fs].T).astype(np.float16),
            "wv": np.ascontiguousarray(np.asarray(v_W, np.float32)[fs].T).astype(np.float16),
            "wo": np.ascontiguousarray(np.asarray(o_W, np.float32)[:, fs].T).astype(np.float16),
        })
    return in_maps


def kernel(x, q_W, k_W, v_W, o_W, trace=False):
    nc = _get_nc()
    in_maps = make_in_maps(x, q_W, k_W, v_W, o_W)
    res = run_bass_kernel_spmd(nc, in_maps, core_ids=list(range(8)),
                               trace=trace)
    _CACHE["last_results"] = res
    ys = [res.results[c]["y"].astype(np.float32) for c in range(8)]
    out = np.stack([
        ys[0] + ys[1] + ys[2] + ys[3],
        ys[4] + ys[5] + ys[6] + ys[7],
    ]).astype(np.float32)
    return out


# revision 6
# speedup vs baseline: 1.0992x; 1.0129x over previous
"""Multi-head causal attention (B=2, S=2048, D=1024, H=16, hd=64) on 8 TRN2
NeuronCores.

Sharding: 2-way batch x 4-way head tensor parallel. Core c handles batch
c//4 and heads 4*(c%4) .. 4*(c%4)+3 (a 256-column feature slice of the QKV
projections / 256 rows of o_W). Each core computes a full [2048, 1024]
partial of its batch's output; the host sums the 4 partials per batch.

v2 structure (all matmuls fp16, fp32 PSUM accumulate):
  - Input DMA split across sync+gpsimd+vector queues, ordered so the
    first Q-projection tile's operands (wq + x tokens 0:512) land first.
  - Q/K/V projection for token chunk t is emitted interleaved into the
    attention stream of query chunk t-1, so the ACT engine (exp) gets
    work within ~5us of kernel start and the PE never drains.
  - Attention per (query chunk j, head pair): scores transposed
    St[k, q] = Kt.T @ Qt, exp via ACT (scale fused), causal mask via
    gpsimd affine_select, AV row-major with a ones-column appended to V
    so the same matmul produces the softmax denominator.
  - Output projection from PE-transposed AO; y stored fp16 (the host
    sums 4 fp16 partials per batch in fp32).
  - Optionally (U8=True) the exp output u is stored fp8e4 (exp shifted
    by -2ln2 to fit e4m3 range; the shift cancels in the softmax ratio)
    which halves the AV stationary LDWEIGHTS traffic.
"""

import numpy as np

import concourse.mybir as mybir
import concourse.tile as tile
from concourse import bacc
from concourse.bass_utils import run_bass_kernel_spmd

F32 = mybir.dt.float32
F16 = mybir.dt.float16
F8 = mybir.dt.float8e4

S = 2048          # tokens per batch (= per core)
D = 1024          # model dim
HD = 64           # head dim
CORE_HEADS = 4    # heads per core
CF = CORE_HEADS * HD  # feature columns per core (256)
QC = 512          # query chunk (QK/exp granularity)
KC = 128          # key chunk
NQ = S // QC      # 4 query chunks
NK = S // KC      # 16 key chunks
ND = D // 128     # 8 contraction chunks

U8 = False        # fp8e4 exp output (AV stationary)

_CACHE = {}


def build_nc(u8=U8):
    udt = F8 if u8 else F16
    ubias = float(-2.0 * np.log(2.0)) if u8 else 0.0
    nc = bacc.Bacc()
    xT = nc.dram_tensor("xT", [D, S], F16, kind="ExternalInput")
    wq = nc.dram_tensor("wq", [D, CF], F16, kind="ExternalInput")
    wk = nc.dram_tensor("wk", [D, CF], F16, kind="ExternalInput")
    wv = nc.dram_tensor("wv", [D, CF], F16, kind="ExternalInput")
    wo = nc.dram_tensor("wo", [CF, D], F16, kind="ExternalInput")
    y = nc.dram_tensor("y", [S, D], F16, kind="ExternalOutput")

    with tile.TileContext(nc) as tc:
        with (
            tc.tile_pool(name="big", bufs=1) as big,
            tc.tile_pool(name="w", bufs=1) as wpool,
            tc.tile_pool(name="u", bufs=34) as upool,
            tc.tile_pool(name="aoq", bufs=16) as aoqpool,
            tc.tile_pool(name="aot", bufs=3) as aotpool,
            tc.tile_pool(name="sm", bufs=8) as smpool,
            tc.tile_pool(name="ost", bufs=4) as ostpool,
            tc.tile_pool(name="ps", bufs=2, space="PSUM") as psp,
            tc.tile_pool(name="pav", bufs=2, space="PSUM") as pavp,
            tc.tile_pool(name="pt", bufs=2, space="PSUM") as ptp,
        ):
            # ---- constants ----
            ident = wpool.tile([128, 128], F16, tag="ident")
            nc.gpsimd.memset(ident[:], 0.0)
            nc.gpsimd.affine_select(
                out=ident[:], in_=ident[:],
                compare_op=mybir.AluOpType.not_equal, fill=1.0,
                base=0, channel_multiplier=1, pattern=[[-1, 128]],
            )

            # ---- weight + activation loads ----
            # Few large DMAs (issue costs ~600ns each), ordered so the Q
            # projection of token chunk 0 can start first: x tokens 0:512
            # stream on the gpsimd queue while wq streams on sync; the
            # remaining token chunks arrive one attention-unit ahead of the
            # projection that consumes them.
            wq_sb = wpool.tile([128, ND, CF], F16, tag="wq")
            wk_sb = wpool.tile([128, ND, CF], F16, tag="wk")
            wv_sb = wpool.tile([128, ND, CF], F16, tag="wv")
            wo_sb = wpool.tile([128, 2, D], F16, tag="wo")
            xs = big.tile([128, ND, S], F16, tag="xs")
            xTv = xT.rearrange("(n p) m -> p n m", p=128)

            nc.gpsimd.dma_start(xs[:, 0:4, 0:QC], xTv[:, 0:4, 0:QC])
            nc.sync.dma_start(wq_sb[:], wq.rearrange("(n p) m -> p n m", p=128))
            nc.gpsimd.dma_start(xs[:, 4:8, 0:QC], xTv[:, 4:8, 0:QC])
            nc.sync.dma_start(wk_sb[:], wk.rearrange("(n p) m -> p n m", p=128))
            nc.sync.dma_start(wv_sb[:], wv.rearrange("(n p) m -> p n m", p=128))
            nc.gpsimd.dma_start(xs[:, :, QC:2 * QC], xTv[:, :, QC:2 * QC])
            nc.sync.dma_start(xs[:, :, 2 * QC:3 * QC], xTv[:, :, 2 * QC:3 * QC])
            nc.gpsimd.dma_start(wo_sb[:], wo.rearrange("(b p) n -> p b n", p=128))
            nc.gpsimd.dma_start(xs[:, :, 3 * QC:], xTv[:, :, 3 * QC:])

            # ---- Q/K/V projections ----
            # qt/kt: [128, 2, S]: partition = feat % 128 (2 heads), block =
            # feat // 128 (head pair), col = token.
            qt = big.tile([128, 2, S], F16, tag="qt")
            kt = big.tile([128, 2, S], F16, tag="kt")
            v_sb = big.tile([128, NK, CORE_HEADS * (HD + 1)], F16, tag="v")
            nc.vector.memset(
                v_sb[:].rearrange("p n (h c) -> p n h c", c=HD + 1)[:, :, :, HD:],
                1.0,
            )

            def emit_proj(t):
                """Generator: QKV projection matmuls for token chunk t."""
                for f in range(2):
                    ps_q = ptp.tile([128, QC], F32, tag="t", name=f"pq{t}_{f}")
                    ps_k = ptp.tile([128, QC], F32, tag="t", name=f"pk{t}_{f}")
                    with nc.named_scope("mm_projqk"):
                        for k in range(ND):
                            nc.tensor.matmul(
                                ps_q[:],
                                wq_sb[:, k, 128 * f:128 * (f + 1)],
                                xs[:, k, QC * t:QC * (t + 1)],
                                start=(k == 0), stop=(k == ND - 1),
                            )
                    yield
                    nc.vector.tensor_copy(qt[:, f, QC * t:QC * (t + 1)], ps_q[:])
                    with nc.named_scope("mm_projqk"):
                        for k in range(ND):
                            nc.tensor.matmul(
                                ps_k[:],
                                wk_sb[:, k, 128 * f:128 * (f + 1)],
                                xs[:, k, QC * t:QC * (t + 1)],
                                start=(k == 0), stop=(k == ND - 1),
                            )
                    yield
                    nc.vector.tensor_copy(kt[:, f, QC * t:QC * (t + 1)], ps_k[:])
                for tt in range(4 * t, 4 * t + 4):
                    ps = ptp.tile([128, CF], F32, tag="t")
                    with nc.named_scope("mm_projv"):
                        for k in range(ND):
                            nc.tensor.matmul(
                                ps[:],
                                xs[:, k, KC * tt:KC * (tt + 1)],
                                wv_sb[:, k, :],
                                start=(k == 0), stop=(k == ND - 1),
                            )
                    yield
                    nc.vector.tensor_copy(
                        v_sb[:, tt, :].rearrange("p (h c) -> p h c", c=HD + 1)[:, :, :HD],
                        ps[:].rearrange("p (h c) -> p h c", c=HD),
                    )

            # ---- attention + output projection ----
            # Software-pipelined across (query-chunk, head-pair) units: the
            # AV matmuls of unit k-1 (pure PE work) are interleaved with the
            # QK+exp phase of unit k (ACT-paced) so the PE never waits on
            # the scalar engine. The projection of token chunk j+1 is pumped
            # into the same stream.
            units = [(j, pair) for j in range(NQ) for pair in range(2)]
            us = {}
            ao_q = {}

            def emit_A(unit, i):
                # Both heads' scores land in one 2-bank PSUM tile so a
                # single ACTIVATE (and a single affine_select) covers the
                # pair — halves the fixed ACT pipeline overhead.
                j, pair = unit
                t = i - 4 * j
                qo = max(0, KC * t)
                w = QC - qo
                ps_s = psp.tile([128, 2, QC], F32, tag="s", name=f"s{j}_{pair}_{i}")
                for hx, h in enumerate((2 * pair, 2 * pair + 1)):
                    hp = 64 * (h % 2)
                    with nc.named_scope("mm_qk"):
                        nc.tensor.matmul(
                            ps_s[:, hx, 0:w],
                            kt[hp:hp + 64, pair, KC * i:KC * (i + 1)],
                            qt[hp:hp + 64, pair, QC * j + qo:QC * (j + 1)],
                            start=True, stop=True,
                            skip_group_check=True,
                        )
                u = upool.tile([128, 2, w], udt, tag="u", name=f"u{j}_{pair}_{i}")
                nc.scalar.activation(
                    u[:], ps_s[:, :, 0:w],
                    mybir.ActivationFunctionType.Exp, scale=0.125, bias=ubias,
                )
                if t >= 0:
                    nc.gpsimd.affine_select(
                        out=u[:, :, 0:KC], in_=u[:, :, 0:KC],
                        compare_op=mybir.AluOpType.is_ge, fill=0.0,
                        base=0, channel_multiplier=-1,
                        pattern=[[0, 2], [1, KC]],
                    )
                us[unit, i] = (u, qo)

            def emit_out_qq(j, qq):
                """Transpose + output projection + store for one 128-token
                query subchunk (all 4 heads of this core)."""
                aot = aotpool.tile([128, 2, KC], F16, tag="aot",
                                   name=f"aot{j}_{qq}")
                for b in range(2):
                    ps_t = ptp.tile([128, 128], F16, tag="t",
                                    name=f"pt{j}_{qq}_{b}")
                    with nc.named_scope("mm_tpose"):
                        nc.tensor.transpose(
                            ps_t[:], ao_q[j][qq][:, 128 * b:128 * (b + 1)],
                            ident[:],
                        )
                    nc.vector.tensor_copy(aot[:, b, :], ps_t[:])
                ps_o = [ptp.tile([128, QC], F32, tag="t",
                                 name=f"po{j}_{qq}_{n}")
                        for n in range(2)]
                with nc.named_scope("mm_oproj"):
                    for b in range(2):
                        for n in range(2):
                            nc.tensor.matmul(
                                ps_o[n][:],
                                aot[:, b, :],
                                wo_sb[:, b, 512 * n:512 * (n + 1)],
                                start=(b == 0), stop=(b == 1),
                            )
                ost = ostpool.tile([128, D], F16, tag="ost",
                                   name=f"ost{j}_{qq}")
                for n in range(2):
                    nc.vector.tensor_copy(ost[:, 512 * n:512 * (n + 1)],
                                          ps_o[n][:])
                nc.sync.dma_start(
                    y[QC * j + KC * qq:QC * j + KC * (qq + 1), :], ost[:],
                )

            def emit_B(unit):
                """Generator: AV matmuls for one unit, yielding after each
                key-chunk step; norms at each query-subchunk's end. For the
                second head pair, the finished subchunk's output projection
                is emitted immediately so y streams out incrementally.
                One PSUM bank per accumulation group (bank-granular
                start/stop semantics)."""
                j, pair = unit
                nk = 4 * j + 4
                if j not in ao_q:
                    ao_q[j] = [aoqpool.tile([128, CF], F16, tag="aoq",
                                            name=f"ao_q{j}_{qq}")
                               for qq in range(4)]
                for qq in range(4):
                    for hx, h in enumerate((2 * pair, 2 * pair + 1)):
                        av = pavp.tile([128, HD + 1], F32, tag="av",
                                       name=f"av{j}_{h}_{qq}")
                        last = 4 * j + qq
                        with nc.named_scope("mm_av"):
                            for i in range(last + 1):
                                u, qo = us[unit, i]
                                nc.tensor.matmul(
                                    av[:],
                                    u[:, hx, KC * qq - qo:KC * (qq + 1) - qo],
                                    v_sb[:, i, 65 * h:65 * h + 65],
                                    start=(i == 0), stop=(i == last),
                                )
                                yield
                        rec = smpool.tile([128, 1], F32, tag="rec",
                                          name=f"rec{j}_{h}_{qq}")
                        nc.vector.reciprocal(rec[:], av[:, HD:HD + 1])
                        nc.vector.tensor_scalar_mul(
                            ao_q[j][qq][:, HD * h:HD * (h + 1)],
                            av[:, 0:HD], rec[:],
                        )
                    if pair == 1:
                        emit_out_qq(j, qq)
                    yield
                for i in range(nk):
                    us.pop((unit, i), None)

            def pump(gen):
                if gen is not None and next(gen, "done") == "done":
                    return None
                return gen

            # token chunk 0 projection runs standalone; chunk j+1 is pumped
            # into query chunk j's attention stream.
            for _ in emit_proj(0):
                pass

            prev_gen = None
            prev_unit = None
            proj_gen = None
            for unit in units:
                j, pair = unit
                if pair == 0 and j + 1 < NQ:
                    proj_gen = emit_proj(j + 1)
                nk_c = 4 * j + 4
                for ii in range(nk_c):
                    emit_A(unit, ii)
                    proj_gen = pump(proj_gen)
                    if prev_gen is not None:
                        for _ in range(2):
                            next(prev_gen, None)
                if prev_gen is not None:
                    for _ in prev_gen:
                        proj_gen = pump(proj_gen)
                prev_gen = emit_B(unit)
                prev_unit = unit
            for _ in prev_gen:
                proj_gen = pump(proj_gen)
    nc.compile()
    return nc


def _get_nc():
    if "nc" not in _CACHE:
        _CACHE["nc"] = build_nc()
    return _CACHE["nc"]


def make_in_maps(x, q_W, k_W, v_W, o_W):
    x = np.asarray(x, dtype=np.float32)
    in_maps = []
    xTs = [np.ascontiguousarray(x[b].T).astype(np.float16) for b in range(2)]
    for c in range(8):
        b, g = c // 4, c % 4
        fs = slice(CF * g, CF * (g + 1))
        in_maps.append({
            "xT": xTs[b],
            "wq": np.ascontiguousarray(np.asarray(q_W, np.float32)[fs].T).astype(np.float16),
            "wk": np.ascontiguousarray(np.asarray(k_W, np.float32)[fs].T).astype(np.float16),
            "wv": np.ascontiguousarray(np.asarray(v_W, np.float32)[fs].T).astype(np.float16),
            "wo": np.ascontiguousarray(np.asarray(o_W, np.float32)[:, fs].T).astype(np.float16),
        })
    return in_maps


def kernel(x, q_W, k_W, v_W, o_W, trace=False):
    nc = _get_nc()
    in_maps = make_in_maps(x, q_W, k_W, v_W, o_W)
    res = run_bass_kernel_spmd(nc, in_maps, core_ids=list(range(8)),
                               trace=trace)
    _CACHE["last_results"] = res
    ys = [res.results[c]["y"].astype(np.float32) for c in range(8)]
    out = np.stack([
        ys[0] + ys[1] + ys[2] + ys[3],
        ys[4] + ys[5] + ys[6] + ys[7],
    ]).astype(np.float32)
    return out


# revision 8
# speedup vs baseline: 1.1380x; 1.0353x over previous
"""Multi-head causal attention (B=2, S=2048, D=1024, H=16, hd=64) on 8 TRN2
NeuronCores.

Sharding: 2-way batch x 4-way head tensor parallel. Core c handles batch
c//4 and heads 4*(c%4) .. 4*(c%4)+3 (a 256-column feature slice of the QKV
projections / 256 rows of o_W). Each core computes a full [2048, 1024]
partial of its batch's output; the host sums the 4 partials per batch.

v2 structure (all matmuls fp16, fp32 PSUM accumulate):
  - Input DMA split across sync+gpsimd+vector queues, ordered so the
    first Q-projection tile's operands (wq + x tokens 0:512) land first.
  - Q/K/V projection for token chunk t is emitted interleaved into the
    attention stream of query chunk t-1, so the ACT engine (exp) gets
    work within ~5us of kernel start and the PE never drains.
  - Attention per (query chunk j, head pair): scores transposed
    St[k, q] = Kt.T @ Qt, exp via ACT (scale fused), causal mask via
    gpsimd affine_select, AV row-major with a ones-column appended to V
    so the same matmul produces the softmax denominator.
  - Output projection from PE-transposed AO; y stored fp16 (the host
    sums 4 fp16 partials per batch in fp32).
  - Optionally (U8=True) the exp output u is stored fp8e4 (exp shifted
    by -2ln2 to fit e4m3 range; the shift cancels in the softmax ratio)
    which halves the AV stationary LDWEIGHTS traffic.
"""

import numpy as np

import concourse.mybir as mybir
import concourse.tile as tile
from concourse import bacc
from concourse.bass_utils import run_bass_kernel_spmd

F32 = mybir.dt.float32
F16 = mybir.dt.float16
F8 = mybir.dt.float8e4

S = 2048          # tokens per batch (= per core)
D = 1024          # model dim
HD = 64           # head dim
CORE_HEADS = 4    # heads per core
CF = CORE_HEADS * HD  # feature columns per core (256)
QC = 512          # query chunk (QK/exp granularity)
KC = 128          # key chunk
NQ = S // QC      # 4 query chunks
NK = S // KC      # 16 key chunks
ND = D // 128     # 8 contraction chunks

U8 = False        # fp8e4 exp output (AV stationary)

_CACHE = {}


def build_nc(u8=U8):
    udt = F8 if u8 else F16
    ubias = float(-2.0 * np.log(2.0)) if u8 else 0.0
    nc = bacc.Bacc()
    xT = nc.dram_tensor("xT", [D, S], F16, kind="ExternalInput")
    wq = nc.dram_tensor("wq", [D, CF], F16, kind="ExternalInput")
    wk = nc.dram_tensor("wk", [D, CF], F16, kind="ExternalInput")
    wv = nc.dram_tensor("wv", [D, CF], F16, kind="ExternalInput")
    wo = nc.dram_tensor("wo", [CF, D], F16, kind="ExternalInput")
    y = nc.dram_tensor("y", [S, D], F16, kind="ExternalOutput")

    with tile.TileContext(nc) as tc:
        with (
            tc.tile_pool(name="big", bufs=1) as big,
            tc.tile_pool(name="w", bufs=1) as wpool,
            tc.tile_pool(name="u", bufs=34) as upool,
            tc.tile_pool(name="aoq", bufs=16) as aoqpool,
            tc.tile_pool(name="aot", bufs=3) as aotpool,
            tc.tile_pool(name="sm", bufs=8) as smpool,
            tc.tile_pool(name="ost", bufs=4) as ostpool,
            tc.tile_pool(name="ps", bufs=2, space="PSUM") as psp,
            tc.tile_pool(name="pav", bufs=2, space="PSUM") as pavp,
            tc.tile_pool(name="pt", bufs=2, space="PSUM") as ptp,
        ):
            # ---- constants ----
            ident = wpool.tile([128, 128], F16, tag="ident")
            nc.gpsimd.memset(ident[:], 0.0)
            nc.gpsimd.affine_select(
                out=ident[:], in_=ident[:],
                compare_op=mybir.AluOpType.not_equal, fill=1.0,
                base=0, channel_multiplier=1, pattern=[[-1, 128]],
            )

            # ---- weight + activation loads ----
            # Few large DMAs (issue costs ~600ns each), ordered so the Q
            # projection of token chunk 0 can start first: x tokens 0:512
            # stream on the gpsimd queue while wq streams on sync; the
            # remaining token chunks arrive one attention-unit ahead of the
            # projection that consumes them.
            wq_sb = wpool.tile([128, ND, CF], F16, tag="wq")
            wk_sb = wpool.tile([128, ND, CF], F16, tag="wk")
            wv_sb = wpool.tile([128, ND, CF], F16, tag="wv")
            wo_sb = wpool.tile([128, 2, D], F16, tag="wo")
            xs = big.tile([128, ND, S], F16, tag="xs")
            xTv = xT.rearrange("(n p) m -> p n m", p=128)

            nc.sync.dma_start(wq_sb[:], wq.rearrange("(n p) m -> p n m", p=128))
            nc.scalar.dma_start(xs[:, 4:8, 0:QC], xTv[:, 4:8, 0:QC])
            nc.sync.dma_start(xs[:, 0:4, 0:QC], xTv[:, 0:4, 0:QC])
            nc.scalar.dma_start(wk_sb[:], wk.rearrange("(n p) m -> p n m", p=128))
            nc.sync.dma_start(wv_sb[:], wv.rearrange("(n p) m -> p n m", p=128))
            nc.scalar.dma_start(xs[:, :, QC:2 * QC], xTv[:, :, QC:2 * QC])
            nc.sync.dma_start(xs[:, :, 2 * QC:3 * QC], xTv[:, :, 2 * QC:3 * QC])
            nc.scalar.dma_start(wo_sb[:], wo.rearrange("(b p) n -> p b n", p=128))
            nc.sync.dma_start(xs[:, :, 3 * QC:], xTv[:, :, 3 * QC:])

            # ---- Q/K/V projections ----
            # qt/kt: [128, 2, S]: partition = feat % 128 (2 heads), block =
            # feat // 128 (head pair), col = token.
            qt = big.tile([128, 2, S], F16, tag="qt")
            kt = big.tile([128, 2, S], F16, tag="kt")
            v_sb = big.tile([128, NK, CORE_HEADS * (HD + 1)], F16, tag="v")
            nc.vector.memset(
                v_sb[:].rearrange("p n (h c) -> p n h c", c=HD + 1)[:, :, :, HD:],
                1.0,
            )

            def emit_proj(t):
                """Generator: QKV projection matmuls for token chunk t."""
                for f in range(2):
                    ps_q = ptp.tile([128, QC], F32, tag="t", name=f"pq{t}_{f}")
                    ps_k = ptp.tile([128, QC], F32, tag="t", name=f"pk{t}_{f}")
                    with nc.named_scope("mm_projqk"):
                        for k in range(ND):
                            nc.tensor.matmul(
                                ps_q[:],
                                wq_sb[:, k, 128 * f:128 * (f + 1)],
                                xs[:, k, QC * t:QC * (t + 1)],
                                start=(k == 0), stop=(k == ND - 1),
                            )
                    yield
                    nc.vector.tensor_copy(qt[:, f, QC * t:QC * (t + 1)], ps_q[:])
                    with nc.named_scope("mm_projqk"):
                        for k in range(ND):
                            nc.tensor.matmul(
                                ps_k[:],
                                wk_sb[:, k, 128 * f:128 * (f + 1)],
                                xs[:, k, QC * t:QC * (t + 1)],
                                start=(k == 0), stop=(k == ND - 1),
                            )
                    yield
                    nc.vector.tensor_copy(kt[:, f, QC * t:QC * (t + 1)], ps_k[:])
                for tt in range(4 * t, 4 * t + 4):
                    ps = ptp.tile([128, CF], F32, tag="t")
                    with nc.named_scope("mm_projv"):
                        for k in range(ND):
                            nc.tensor.matmul(
                                ps[:],
                                xs[:, k, KC * tt:KC * (tt + 1)],
                                wv_sb[:, k, :],
                                start=(k == 0), stop=(k == ND - 1),
                            )
                    yield
                    nc.vector.tensor_copy(
                        v_sb[:, tt, :].rearrange("p (h c) -> p h c", c=HD + 1)[:, :, :HD],
                        ps[:].rearrange("p (h c) -> p h c", c=HD),
                    )

            # ---- attention + output projection ----
            # Software-pipelined across (query-chunk, head-pair) units: the
            # AV matmuls of unit k-1 (pure PE work) are interleaved with the
            # QK+exp phase of unit k (ACT-paced) so the PE never waits on
            # the scalar engine. The projection of token chunk j+1 is pumped
            # into the same stream.
            units = [(j, pair) for j in range(NQ) for pair in range(2)]
            us = {}
            ao_q = {}

            def emit_A(unit, i):
                # Both heads' scores land in one 2-bank PSUM tile so a
                # single ACTIVATE (and a single affine_select) covers the
                # pair — halves the fixed ACT pipeline overhead.
                j, pair = unit
                t = i - 4 * j
                qo = max(0, KC * t)
                w = QC - qo
                ps_s = psp.tile([128, 2, QC], F32, tag="s", name=f"s{j}_{pair}_{i}")
                for hx, h in enumerate((2 * pair, 2 * pair + 1)):
                    hp = 64 * (h % 2)
                    with nc.named_scope("mm_qk"):
                        nc.tensor.matmul(
                            ps_s[:, hx, 0:w],
                            kt[hp:hp + 64, pair, KC * i:KC * (i + 1)],
                            qt[hp:hp + 64, pair, QC * j + qo:QC * (j + 1)],
                            start=True, stop=True,
                            skip_group_check=True,
                        )
                u = upool.tile([128, 2, w], udt, tag="u", name=f"u{j}_{pair}_{i}")
                nc.scalar.activation(
                    u[:], ps_s[:, :, 0:w],
                    mybir.ActivationFunctionType.Exp, scale=0.125, bias=ubias,
                )
                if t >= 0:
                    nc.gpsimd.affine_select(
                        out=u[:, :, 0:KC], in_=u[:, :, 0:KC],
                        compare_op=mybir.AluOpType.is_ge, fill=0.0,
                        base=0, channel_multiplier=-1,
                        pattern=[[0, 2], [1, KC]],
                    )
                us[unit, i] = (u, qo)

            def emit_out_qq(j, qq):
                """Transpose + output projection + store for one 128-token
                query subchunk (all 4 heads of this core)."""
                aot = aotpool.tile([128, 2, KC], F16, tag="aot",
                                   name=f"aot{j}_{qq}")
                for b in range(2):
                    ps_t = ptp.tile([128, 128], F16, tag="t",
                                    name=f"pt{j}_{qq}_{b}")
                    with nc.named_scope("mm_tpose"):
                        nc.tensor.transpose(
                            ps_t[:], ao_q[j][qq][:, 128 * b:128 * (b + 1)],
                            ident[:],
                        )
                    nc.vector.tensor_copy(aot[:, b, :], ps_t[:])
                ps_o = [ptp.tile([128, QC], F32, tag="t",
                                 name=f"po{j}_{qq}_{n}")
                        for n in range(2)]
                with nc.named_scope("mm_oproj"):
                    for b in range(2):
                        for n in range(2):
                            nc.tensor.matmul(
                                ps_o[n][:],
                                aot[:, b, :],
                                wo_sb[:, b, 512 * n:512 * (n + 1)],
                                start=(b == 0), stop=(b == 1),
                            )
                ost = ostpool.tile([128, D], F16, tag="ost",
                                   name=f"ost{j}_{qq}")
                for n in range(2):
                    nc.vector.tensor_copy(ost[:, 512 * n:512 * (n + 1)],
                                          ps_o[n][:])
                # mid-stream chunks ride the slow gpsimd queue; the final
                # query chunk uses the fast hwdge queues to keep the tail
                # short (scalar's exp work is finished by then).
                if j < NQ - 1:
                    eng = nc.sync if qq % 2 == 0 else nc.gpsimd
                else:
                    eng = nc.sync if qq % 2 == 0 else nc.scalar
                eng.dma_start(
                    y[QC * j + KC * qq:QC * j + KC * (qq + 1), :], ost[:],
                )

            def emit_B(unit):
                """Generator: AV matmuls for one unit, yielding after each
                key-chunk step; norms at each query-subchunk's end. For the
                second head pair, the finished subchunk's output projection
                is emitted immediately so y streams out incrementally.
                One PSUM bank per accumulation group (bank-granular
                start/stop semantics)."""
                j, pair = unit
                nk = 4 * j + 4
                if j not in ao_q:
                    ao_q[j] = [aoqpool.tile([128, CF], F16, tag="aoq",
                                            name=f"ao_q{j}_{qq}")
                               for qq in range(4)]
                for qq in range(4):
                    for hx, h in enumerate((2 * pair, 2 * pair + 1)):
                        av = pavp.tile([128, HD + 1], F32, tag="av",
                                       name=f"av{j}_{h}_{qq}")
                        last = 4 * j + qq
                        with nc.named_scope("mm_av"):
                            for i in range(last + 1):
                                u, qo = us[unit, i]
                                nc.tensor.matmul(
                                    av[:],
                                    u[:, hx, KC * qq - qo:KC * (qq + 1) - qo],
                                    v_sb[:, i, 65 * h:65 * h + 65],
                                    start=(i == 0), stop=(i == last),
                                )
                                yield
                        rec = smpool.tile([128, 1], F32, tag="rec",
                                          name=f"rec{j}_{h}_{qq}")
                        nc.vector.reciprocal(rec[:], av[:, HD:HD + 1])
                        nc.vector.tensor_scalar_mul(
                            ao_q[j][qq][:, HD * h:HD * (h + 1)],
                            av[:, 0:HD], rec[:],
                        )
                    if pair == 1:
                        emit_out_qq(j, qq)
                    yield
                for i in range(nk):
                    us.pop((unit, i), None)

            def pump(gen):
                if gen is not None and next(gen, "done") == "done":
                    return None
                return gen

            # token chunk 0 projection runs standalone; chunk j+1 is pumped
            # into query chunk j's attention stream.
            for _ in emit_proj(0):
                pass

            prev_gen = None
            prev_unit = None
            proj_gen = None
            for unit in units:
                j, pair = unit
                if pair == 0 and j + 1 < NQ:
                    proj_gen = emit_proj(j + 1)
                nk_c = 4 * j + 4
                for ii in range(nk_c):
                    emit_A(unit, ii)
                    proj_gen = pump(proj_gen)
                    if prev_gen is not None:
                        for _ in range(2):
                            next(prev_gen, None)
                if prev_gen is not None:
                    for _ in prev_gen:
                        proj_gen = pump(proj_gen)
                prev_gen = emit_B(unit)
                prev_unit = unit
            for _ in prev_gen:
                proj_gen = pump(proj_gen)
    nc.compile()
    return nc


def _get_nc():
    if "nc" not in _CACHE:
        _CACHE["nc"] = build_nc()
    return _CACHE["nc"]


def make_in_maps(x, q_W, k_W, v_W, o_W):
    x = np.asarray(x, dtype=np.float32)
    in_maps = []
    xTs = [np.ascontiguousarray(x[b].T).astype(np.float16) for b in range(2)]
    for c in range(8):
        b, g = c // 4, c % 4
        fs = slice(CF * g, CF * (g + 1))
        in_maps.append({
            "xT": xTs[b],
            "wq": np.ascontiguousarray(np.asarray(q_W, np.float32)[fs].T).astype(np.float16),
            "wk": np.ascontiguousarray(np.asarray(k_W, np.float32)[fs].T).astype(np.float16),
            "wv": np.ascontiguousarray(np.asarray(v_W, np.float32)[fs].T).astype(np.float16),
            "wo": np.ascontiguousarray(np.asarray(o_W, np.float32)[:, fs].T).astype(np.float16),
        })
    return in_maps


def kernel(x, q_W, k_W, v_W, o_W, trace=False):
    nc = _get_nc()
    in_maps = make_in_maps(x, q_W, k_W, v_W, o_W)
    res = run_bass_kernel_spmd(nc, in_maps, core_ids=list(range(8)),
                               trace=trace)
    _CACHE["last_results"] = res
    ys = [res.results[c]["y"].astype(np.float32) for c in range(8)]
    out = np.stack([
        ys[0] + ys[1] + ys[2] + ys[3],
        ys[4] + ys[5] + ys[6] + ys[7],
    ]).astype(np.float32)
    return out
